# revision 1
# baseline (speedup 1.0000x reference)
"""Causal self-attention on 8 NeuronCores (Bass/Tile, fp32r matmuls).

Sharding: tensor-parallel over heads x data-parallel over batch.
  core c -> batch b = c//4, heads 4g..4g+3 where g = c%4.
Each core computes q,k,v for its 4 heads (over its batch's 2048 tokens),
causal softmax attention in transposed-score layout [k, q] (denominator via
an extra ones-column on v), and the partial output projection over its 256
head-channels. Host sums the 4 partials per batch and adds b_proj.

Matmuls run as float32r (full PE rate at N>=256, ~1e-4 relative rounding);
the attention probabilities p and values v are bf16 (DVE 2x/4x modes; the
softmax numerator and denominator use the same rounded p, so the error
largely cancels). The 1/sqrt(d) scale is folded into W_k/b_k on the host.

The per-512-token stripes are emitted interleaved (qkv stripe ti, then
attention stripe qi=ti) so the Tile scheduler overlaps PE-heavy projection
work with ACT-heavy softmax work; the output projection is emitted last so
its PE work fills the ACT-bound tail of the late (long) attention stripes.
Diagonal score blocks are narrowed to skip fully-masked columns.
"""

import os
import sys

for _p in ("/opt/trn_rl_repo", "/opt/pypackages"):
    if os.path.isdir(_p) and _p not in sys.path:
        sys.path.append(_p)

import numpy as np

import concourse.bass as bass
import concourse.tile as tile
import concourse.mybir as mybir
from concourse import bacc
from concourse.bass_utils import run_bass_kernel_spmd

B, T, C = 2, 2048, 1024
H = 16            # total heads
D = 64            # head dim
HPC = 4           # heads per core
CH = HPC * D      # 256 channels per core
N_CORES = 8

f32 = mybir.dt.float32
f32r = mybir.dt.float32r
bf16 = mybir.dt.bfloat16
ts = bass.ts

_COMPILED = None


def _build():
    nc = bacc.Bacc("TRN2", target_bir_lowering=False, debug=False,
                   num_devices=N_CORES)

    xT = nc.dram_tensor("xT", [C, T], f32, kind="ExternalInput").ap()
    wt = nc.dram_tensor("wt", [C, 3 * CH], f32, kind="ExternalInput").ap()
    wpt = nc.dram_tensor("wpt", [CH, C], f32, kind="ExternalInput").ap()
    bqk = nc.dram_tensor("bqk", [128, 4], f32, kind="ExternalInput").ap()
    bvb = nc.dram_tensor("bvb", [128, CH], f32, kind="ExternalInput").ap()
    Sm = nc.dram_tensor("Sm", [128, 1024], f32, kind="ExternalInput").ap()
    out = nc.dram_tensor("out_partial", [T, C], f32, kind="ExternalOutput").ap()

    NT512 = T // 512          # 4   512-token stripes
    NT128 = T // 128          # 16  128-token tiles
    NC128 = C // 128          # 8   contraction tiles

    with tile.TileContext(nc) as tc:
        with tc.tile_pool(name="consts", bufs=1) as consts, \
             tc.tile_pool(name="qkv", bufs=1) as qkv, \
             tc.tile_pool(name="xp", bufs=3) as xp, \
             tc.tile_pool(name="pp", bufs=8) as pp, \
             tc.tile_pool(name="op", bufs=6) as op, \
             tc.tile_pool(name="small", bufs=4) as small, \
             tc.tile_pool(name="ps_big", bufs=2, space="PSUM") as ps_big, \
             tc.tile_pool(name="ps_s", bufs=3, space="PSUM") as ps_s, \
             tc.tile_pool(name="ps_y", bufs=1, space="PSUM") as ps_y, \
             tc.tile_pool(name="ps_o", bufs=2, space="PSUM") as ps_o:

            # ---- constants; DMA emission order is chosen so the first
            #      qk matmul chains of stripe 0 can start as early as
            #      possible: interleave xt(0)[ci] with the qk half of
            #      wt[ci], defer the v-half / masks / wpt ----
            xT_r = xT.rearrange("(o p) t -> p o t", p=128).bitcast(f32r)
            wt_r = wt.rearrange("(o p) f -> p o f", p=128).bitcast(f32r)
            wt_sb = consts.tile([128, NC128, 3 * CH], f32r)
            xt0 = xp.tile([128, NC128, 512], f32r, tag="xt")
            for ci in range(NC128):
                nc.sync.dma_start(xt0[:, ci], xT_r[:, ci, ts(0, 512)])
                nc.sync.dma_start(wt_sb[:, ci, :512], wt_r[:, ci, :512])
            bqk_sb = consts.tile([128, 4], f32)
            nc.sync.dma_start(bqk_sb[:], bqk)
            for ci in range(NC128):
                nc.sync.dma_start(wt_sb[:, ci, 512:], wt_r[:, ci, 512:])
            bvb_sb = consts.tile([128, CH], f32)
            nc.sync.dma_start(bvb_sb[:], bvb)
            S_f = consts.tile([128, 1024], f32)
            nc.sync.dma_start(S_f[:], Sm)
            S_sb = consts.tile([128, 1024], bf16)
            nc.vector.tensor_copy(S_sb[:], S_f[:])

            onecol_f = consts.tile([128, 1], f32)
            nc.vector.memset(onecol_f[:], 1.0)

            # ---- persistent activations ----
            qT = qkv.tile([128, 2, T], f32r)      # [2h*64, slab, t]
            kT = qkv.tile([128, 2, T], f32r)
            vaug = qkv.tile([128, NT128, HPC, D + 1], bf16)  # [t128, ti, h, d|1]
            yT = qkv.tile([128, 2, T], f32r)

            for h in range(HPC):
                nc.vector.tensor_copy(
                    vaug[:, :, h, D:D + 1],
                    onecol_f[:].to_broadcast([128, NT128, 1]))

            for ti in range(NT512):
                # ---------- QKV projection for stripe ti ----------
                if ti == 0:
                    xt = xt0
                else:
                    xt = xp.tile([128, NC128, 512], f32r, tag="xt")
                    for ci in range(NC128):
                        nc.sync.dma_start(xt[:, ci], xT_r[:, ci, ts(ti, 512)])
                for fj in range(4):          # q0 q1 k0 k1
                    ps = ps_big.tile([128, 512], f32, tag="big")
                    for ci in range(NC128):
                        nc.tensor.matmul(
                            ps[:], wt_sb[:, ci, ts(fj, 128)], xt[:, ci, :],
                            start=(ci == 0), stop=(ci == NC128 - 1))
                    dest = qT if fj < 2 else kT
                    nc.vector.tensor_add(
                        out=dest[:, fj % 2, ts(ti, 512)], in0=ps[:],
                        in1=bqk_sb[:, fj:fj + 1].to_broadcast([128, 512]))
                for tj in range(4):
                    pv = ps_big.tile([128, 512], f32, tag="big")
                    for ci in range(NC128):
                        nc.tensor.matmul(
                            pv[:, :CH], xt[:, ci, ts(tj, 128)],
                            wt_sb[:, ci, 512:512 + CH],
                            start=(ci == 0), stop=(ci == NC128 - 1))
                    for h in range(HPC):
                        nc.vector.tensor_add(
                            out=vaug[:, 4 * ti + tj, h, 0:D],
                            in0=pv[:, ts(h, D)],
                            in1=bvb_sb[:, ts(h, D)])

                # ---------- attention stripe qi = ti ----------
                qi = ti
                nk = 4 * qi + 4
                for h in range(HPC):
                    hp, hs = (h % 2) * D, h // 2
                    py = ps_y.tile([D + 1, 512], f32)
                    for ki in range(nk):
                        j = ki - 4 * qi
                        # columns qq < 128*j of this stripe are fully masked
                        q0 = max(0, 128 * j)
                        w = 512 - q0
                        psc = ps_s.tile([128, 512], f32)
                        nc.tensor.matmul(
                            psc[:, q0:],
                            kT[hp:hp + D, hs, ts(ki, 128)],
                            qT[hp:hp + D, hs, bass.ds(512 * qi + q0, w)],
                            start=True, stop=True)
                        p = pp.tile([128, 512], bf16)
                        nc.scalar.activation(
                            p[:, q0:], psc[:, q0:],
                            mybir.ActivationFunctionType.Exp)
                        if j >= 0:  # partial 128 columns need the causal mask
                            nc.vector.tensor_mul(
                                out=p[:, q0:q0 + 128], in0=p[:, q0:q0 + 128],
                                in1=S_sb[:, 384:512])
                        nc.tensor.matmul(
                            py[:, q0:], vaug[:, ki, h, :], p[:, q0:],
                            start=(ki == 0), stop=(ki == nk - 1))
                    # normalize: yT = py[:D] * (1/py[D]) broadcast over d
                    rec = small.tile([1, 512], f32, tag="rec")
                    nc.vector.reciprocal(rec[:], py[D:D + 1, :])
                    bc = small.tile([D, 512], f32, tag="bc")
                    nc.gpsimd.partition_broadcast(bc[:], rec[:], channels=D)
                    nc.vector.tensor_mul(
                        out=yT[hp:hp + D, hs, ts(qi, 512)],
                        in0=py[0:D, :], in1=bc[:])

            wpt_sb = consts.tile([128, 2, C], f32r)
            nc.sync.dma_start(
                wpt_sb[:], wpt.rearrange("(s p) o -> p s o", p=128).bitcast(f32r))

            # ---------- output projection (emitted last so its PE work
            #            fills the ACT-bound tail of late attention stripes) --
            for tg in range(NT128):
                for oi in range(2):
                    po = ps_o.tile([128, 512], f32, tag="po")
                    for s in range(2):
                        nc.tensor.matmul(
                            po[:], yT[:, s, ts(tg, 128)],
                            wpt_sb[:, s, ts(oi, 512)],
                            start=(s == 0), stop=(s == 1))
                    ot = op.tile([128, 512], f32)
                    nc.vector.tensor_copy(ot[:], po[:])
                    nc.sync.dma_start(
                        out[ts(tg, 128), ts(oi, 512)], ot[:])

    nc.compile()
    return nc


def _get_compiled():
    global _COMPILED
    if _COMPILED is None:
        _COMPILED = _build()
    return _COMPILED


def _host_prep(x, W_attn, b_attn, W_proj, b_proj):
    scale = 1.0 / np.sqrt(np.float32(D))
    xTb = [np.ascontiguousarray(x[b].T).astype(np.float32) for b in range(B)]
    Sm = (np.arange(1024, dtype=np.int32)[None, :]
          >= (np.arange(128, dtype=np.int32)[:, None] + 384)).astype(np.float32)
    in_maps = []
    for c in range(N_CORES):
        b, g = divmod(c, 4)
        ch = slice(CH * g, CH * (g + 1))
        Wq = W_attn[ch]
        Wk = W_attn[C:][ch] * scale
        Wv = W_attn[2 * C:][ch]
        wt_c = np.ascontiguousarray(
            np.concatenate([Wq, Wk, Wv], axis=0).T).astype(np.float32)
        bq = b_attn[ch]
        bk = b_attn[C:][ch] * scale
        bv = b_attn[2 * C:][ch]
        bqk_c = np.ascontiguousarray(
            np.concatenate([bq, bk]).reshape(4, 128).T).astype(np.float32)
        bvb_c = np.ascontiguousarray(
            np.broadcast_to(bv[None, :], (128, CH))).astype(np.float32)
        wpt_c = np.ascontiguousarray(W_proj[:, ch].T).astype(np.float32)
        in_maps.append({
            "xT": xTb[b],
            "wt": wt_c,
            "wpt": wpt_c,
            "bqk": bqk_c,
            "bvb": bvb_c,
            "Sm": Sm,
        })
    return in_maps


def kernel(x, W_attn, b_attn, W_proj, b_proj):
    x = np.asarray(x, dtype=np.float32)
    W_attn = np.asarray(W_attn, dtype=np.float32)
    b_attn = np.asarray(b_attn, dtype=np.float32)
    W_proj = np.asarray(W_proj, dtype=np.float32)
    b_proj = np.asarray(b_proj, dtype=np.float32)

    nc = _get_compiled()
    in_maps = _host_prep(x, W_attn, b_attn, W_proj, b_proj)
    res = run_bass_kernel_spmd(nc, in_maps, core_ids=list(range(N_CORES)))

    out = np.empty((B, T, C), dtype=np.float32)
    for b in range(B):
        acc = res.results[4 * b]["out_partial"].copy()
        for g in range(1, 4):
            acc += res.results[4 * b + g]["out_partial"]
        out[b] = acc + b_proj
    return out



# revision 31
# speedup vs baseline: 1.2453x; 1.2453x over previous
"""Causal self-attention on 8 NeuronCores (Bass/Tile).

Sharding: tensor-parallel over heads x data-parallel over batch.
  core c -> batch b = c//4, heads 4g..4g+3 where g = c%4.

v2 design (vs v1 baseline):
- all activations/weights bf16 (halves DMA + DVE traffic; matmuls run at
  1 cyc/row either way).
- scores run as fp8e4 DoubleRow matmuls at 0.5 cyc/row: contraction d=64
  is padded to a 2-tile DoubleRow pair whose second slot is all-zero
  (q/k slot-1 memset once), so out = k.T q exactly. The 1/sqrt(d) scale is
  applied by the Exp activation's scale operand, keeping q/k at their
  natural magnitude for fp8.
- p@v runs transposed-from-v1: out [q=128, d+1] with p as stationary, so
  cost is 65 rows per (q-tile, k-tile) instead of 512 (2x less PE).
- y [tok, ch] is transposed to [ch, tok] for the output projection by PE
  transposes (128x128 bf16, 53ns each) + DVE psum evacuation.
- exp is batched over ki-pairs ([128, 1024] per activation), bias adds are
  batched over fj-pairs / the 4 heads, the causal triangle mask is one
  [128, 4, 128] DVE multiply per diagonal block.
- output partials are written bf16 and summed on host (fp32) with b_proj.
"""

import os
import sys

for _p in ("/opt/trn_rl_repo", "/opt/pypackages"):
    if os.path.isdir(_p) and _p not in sys.path:
        sys.path.append(_p)

import numpy as np
import ml_dtypes

import concourse.bass as bass
import concourse.tile as tile
import concourse.mybir as mybir
from concourse import bacc
from concourse.bass_utils import run_bass_kernel_spmd

B, T, C = 2, 2048, 1024
H = 16            # total heads
D = 64            # head dim
HPC = 4           # heads per core
CH = HPC * D      # 256 channels per core
N_CORES = 8

f32 = mybir.dt.float32
bf16 = mybir.dt.bfloat16
fp8 = mybir.dt.float8e4
ts = bass.ts
ds = bass.ds
DR = mybir.MatmulPerfMode.DoubleRow
Exp = mybir.ActivationFunctionType.Exp

FP8_SCORES = os.environ.get("FP8_SCORES", "1") == "1"

_COMPILED = None


def _build():
    nc = bacc.Bacc("TRN2", target_bir_lowering=False, debug=False,
                   num_devices=N_CORES)

    xT = nc.dram_tensor("xT", [C, T], bf16, kind="ExternalInput").ap()
    wt = nc.dram_tensor("wt", [C, 3 * CH], bf16, kind="ExternalInput").ap()
    wpt = nc.dram_tensor("wpt", [CH, C], bf16, kind="ExternalInput").ap()
    bqk = nc.dram_tensor("bqk", [128, 4], f32, kind="ExternalInput").ap()
    bvb2 = nc.dram_tensor("bvb2", [128, 512], f32, kind="ExternalInput").ap()
    Sm = nc.dram_tensor("Sm", [128, 512], bf16, kind="ExternalInput").ap()
    ident = nc.dram_tensor("ident", [128, 128], bf16, kind="ExternalInput").ap()
    out = nc.dram_tensor("out_partial", [T, C], bf16, kind="ExternalOutput").ap()

    NT512 = T // 512          # 4   512-token stripes
    NT128 = T // 128          # 16  128-token tiles
    NC128 = C // 128          # 8   contraction tiles

    kdt = fp8 if FP8_SCORES else bf16

    xT_r = xT.rearrange("(o p) t -> p o t", p=128)
    wt_r = wt.rearrange("(o p) f -> p o f", p=128)

    with tile.TileContext(nc) as tc:
        with tc.tile_pool(name="consts", bufs=1) as consts, \
             tc.tile_pool(name="qkv", bufs=1) as qkv, \
             tc.tile_pool(name="xp", bufs=4) as xp, \
             tc.tile_pool(name="pp", bufs=10) as pp, \
             tc.tile_pool(name="op", bufs=3) as op, \
             tc.tile_pool(name="small", bufs=4) as small, \
             tc.tile_pool(name="ps_qkv", bufs=2, space="PSUM") as ps_qkv, \
             tc.tile_pool(name="ps_big", bufs=2, space="PSUM") as ps_big, \
             tc.tile_pool(name="ps_pv", bufs=2, space="PSUM") as ps_pv:

            # ---- constants; chunked so stripe-0 chains start after the
            #      first half of wt/x is resident ----
            wt_sb = consts.tile([128, NC128, 3 * CH], bf16)
            xt0 = xp.tile([128, NC128, 512], bf16, tag="xt")
            nc.sync.dma_start(wt_sb[:, :4, :512], wt_r[:, :4, :512])
            nc.sync.dma_start(xt0[:, :4], xT_r[:, :4, ts(0, 512)])
            bqk_sb = consts.tile([128, 4], f32)
            nc.sync.dma_start(bqk_sb[:], bqk)
            nc.sync.dma_start(wt_sb[:, 4:, :512], wt_r[:, 4:, :512])
            nc.sync.dma_start(xt0[:, 4:], xT_r[:, 4:, ts(0, 512)])
            nc.sync.dma_start(wt_sb[:, :, 512:], wt_r[:, :, 512:])
            bvb_sb = consts.tile([128, 2, 4, 64], f32)
            nc.sync.dma_start(
                bvb_sb[:], bvb2.rearrange("p (m h d) -> p m h d", m=2, h=4))
            S4_sb = consts.tile([128, 4, 128], bf16)
            nc.sync.dma_start(S4_sb[:], Sm.rearrange("p (h c) -> p h c", h=4))
            id_sb = consts.tile([128, 128], bf16)
            nc.sync.dma_start(id_sb[:], ident)
            xts = [xt0]
            for ti in range(1, NT512):
                xt = xp.tile([128, NC128, 512], bf16, tag="xt")
                nc.sync.dma_start(xt[:], xT_r[:, :, ts(ti, 512)])
                xts.append(xt)

            # ---- persistent activations ----
            # q/k live in bf16 [part=2heads*64d, slab, t]; stripes >= 1 also
            # get an fp8 copy in DoubleRow layout [part, slot, slab, t] whose
            # slot 1 is all-zero, so the fp8 DoubleRow pair sums to k.T q.
            # Stripe 0 (short attention rows, no error averaging) keeps its
            # scores in bf16.
            qT_bf = qkv.tile([128, 2, T], bf16)
            kT_bf = qkv.tile([128, 2, T], bf16)
            vaug = qkv.tile([128, NT128, HPC, D + 1], bf16)  # [tok, ti, h, d|1]
            y_sb = qkv.tile([128, 2, NT128, 128], bf16)      # [tok, slab, tg, ch']
            yT_sb = qkv.tile([128, 2, T], bf16)              # [ch', slab, tok]
            if FP8_SCORES:
                qT8 = qkv.tile([128, 2, 2, T], fp8)
                kT8 = qkv.tile([128, 2, 2, T], fp8)

            # PE warmup: keep the tensor engine continuously busy from t~0.5us
            # until the first real chains, so the p-state ramp completes and
            # never resets. Uses the pv psum ring, whose first real use is far
            # later, so the ring slot is free again in time.
            wu = consts.tile([128, 512], bf16)
            nc.vector.memset(wu[:], 0.0)
            wu_ps = ps_pv.tile([128, HPC, D + 1], f32, tag="pv")
            for _ in range(45):
                nc.tensor.matmul(wu_ps[:].rearrange("p h d -> p (h d)"),
                                 wu[:, :128], wu[:, :HPC * (D + 1)],
                                 start=True, stop=True)

            if FP8_SCORES:
                nc.gpsimd.memset(qT8[:, 1], 0.0)
                nc.gpsimd.memset(kT8[:, 1], 0.0)
            nc.vector.memset(vaug[:, :, :, D:D + 1], 1.0)

            wpt_sb = consts.tile([128, 2, C], bf16)
            nc.sync.dma_start(
                wpt_sb[:], wpt.rearrange("(s p) o -> p s o", p=128))

            def qkv_fj(ti, fj, dest, slab):
                xt = xts[ti]
                ps = ps_qkv.tile([128, 512], f32, tag="qkv")
                for ci in range(NC128):
                    nc.tensor.matmul(
                        ps[:], wt_sb[:, ci, ts(fj, 128)], xt[:, ci, :],
                        start=(ci == 0), stop=(ci == NC128 - 1))
                nc.vector.tensor_add(
                    out=dest[:, slab, ts(ti, 512)], in0=ps[:],
                    in1=bqk_sb[:, fj:fj + 1].to_broadcast([128, 512]))

            def qkv_v(ti):
                xt = xts[ti]
                for m in range(2):
                    pv2 = ps_qkv.tile([128, 2, 4, 64], f32, tag="qkv")
                    for tj in range(2):
                        for ci in range(NC128):
                            nc.tensor.matmul(
                                pv2[:, tj], xt[:, ci, ts(2 * m + tj, 128)],
                                wt_sb[:, ci, 512:512 + CH],
                                start=(ci == 0), stop=(ci == NC128 - 1))
                    nc.vector.tensor_add(
                        out=vaug[:, 4 * ti + 2 * m:4 * ti + 2 * m + 2, :, 0:D],
                        in0=pv2[:], in1=bvb_sb[:])

            def conv_fp8(ti, src, slab):
                # fp8 DoubleRow copy for later stripes' scores, per slab and
                # off the critical engines (GpSimd).
                if not FP8_SCORES or (ti == 0 and src is qT_bf):
                    return
                dst = qT8 if src is qT_bf else kT8
                nc.gpsimd.tensor_copy(
                    dst[:, 0, slab, ts(ti, 512)], src[:, slab, ts(ti, 512)])

            def qkv_stripe(ti):
                # q slab 0 first: the next stripe's first scores gate on it
                for fj, src, slab in ((0, qT_bf, 0), (2, kT_bf, 0),
                                      (1, qT_bf, 1), (3, kT_bf, 1)):
                    qkv_fj(ti, fj, src, slab)
                    conv_fp8(ti, src, slab)
                qkv_v(ti)

            def emit_pair(qi, pi, p_t, heads):
                """Scores + exp for pair pi of stripe qi, given heads only.
                Masks are NOT emitted here (need all 4 heads done)."""
                use8 = FP8_SCORES and qi > 0
                diag = pi >= 2 * qi
                j0 = 2 * pi - 4 * qi
                if diag:
                    w0 = 512 - 128 * j0
                    offs, q0s = (0, w0), (128 * j0, 128 * (j0 + 1))
                    wtot = w0 + (384 - 128 * j0)
                else:
                    offs, q0s, wtot = (0, 512), (0, 0), 1024
                for h in heads:
                    hp, hs = (h % 2) * D, h // 2
                    psc = ps_big.tile([128, 1024], f32, tag="big")
                    for half in range(2):
                        ki = 2 * pi + half
                        w = 512 - q0s[half]
                        if use8:
                            nc.tensor.matmul(
                                psc[:, ds(offs[half], w)],
                                kT8[hp:hp + D, :, hs, ts(ki, 128)],
                                qT8[hp:hp + D, :, hs,
                                    ds(512 * qi + q0s[half], w)],
                                start=True, stop=True, perf_mode=DR)
                        else:
                            nc.tensor.matmul(
                                psc[:, ds(offs[half], w)],
                                kT_bf[hp:hp + D, hs, ts(ki, 128)],
                                qT_bf[hp:hp + D, hs,
                                      ds(512 * qi + q0s[half], w)],
                                start=True, stop=True)
                    nc.scalar.activation(
                        p_t[:, h, 0:wtot], psc[:, 0:wtot], Exp, scale=0.125)
                return offs if diag else None

            def emit_mask(p_t, offs):
                for half in range(2):
                    nc.vector.tensor_mul(
                        out=p_t[:, :, ds(offs[half], 128)],
                        in0=p_t[:, :, ds(offs[half], 128)], in1=S4_sb[:])

            # stripe 0 fast path: interleave its (bf16) pair-0 scores with the
            # qkv chains so the ACT pipeline starts as early as possible.
            p0_t = pp.tile([128, HPC, 1024], bf16, tag="p")
            qkv_fj(0, 0, qT_bf, 0)
            qkv_fj(0, 2, kT_bf, 0)
            conv_fp8(0, kT_bf, 0)
            with tc.high_priority(offset=8000):
                emit_pair(0, 0, p0_t, (0, 1))
            qkv_fj(0, 1, qT_bf, 1)
            qkv_fj(0, 3, kT_bf, 1)
            conv_fp8(0, kT_bf, 1)
            with tc.high_priority(offset=8000):
                offs0 = emit_pair(0, 0, p0_t, (2, 3))
                emit_mask(p0_t, offs0)
            qkv_v(0)

            for ti in range(NT512):
                # ---------- attention stripe qi = ti ----------
                # p tiles use a packed column layout [128, h, 1024]: for
                # off-diagonal ki-pairs, cols [512*half : 512*half+512]; for
                # diagonal pairs the two ragged blocks [q0:512] are packed
                # back-to-back at [0:w0] and [w0:w0+w1] so exp is one
                # activation per (pair, head).
                qi = ti
                nk = 4 * qi + 4
                npair = nk // 2

                def pcol(ki, qq):
                    # packed p-tile column of query subtile qq for key tile ki
                    j = ki - 4 * qi
                    if j < 0:
                        return 512 * (ki % 2) + 128 * qq
                    half = j % 2
                    off = 0 if half == 0 else 512 - 128 * (j - 1)
                    return off + 128 * (qq - j)

                # scores/exp outrank all other ready work: they are cheap on
                # PE but gate the ACT stream, which is the secondary
                # bottleneck.
                p_tiles = []
                with tc.high_priority(offset=8000):
                    for pi in range(npair):
                        if qi == 0 and pi == 0:
                            p_tiles.append(p0_t)
                            continue
                        p_t = pp.tile([128, HPC, 1024], bf16, tag="p")
                        p_tiles.append(p_t)
                        offs = emit_pair(qi, pi, p_t, range(HPC))
                        if offs is not None:
                            emit_mask(p_t, offs)

                # qkv for the next stripe: gates the next ACT phase, so it
                # goes ahead of this stripe's pv/outproj filler work.
                if ti + 1 < NT512:
                    qkv_stripe(ti + 1)

                for qq in range(4):
                    pv = ps_pv.tile([128, HPC, D + 1], f32, tag="pv")
                    lastki = 4 * qi + qq
                    for h in range(HPC):
                        for ki in range(lastki + 1):
                            nc.tensor.matmul(
                                pv[:, h],
                                p_tiles[ki // 2][:, h, ds(pcol(ki, qq), 128)],
                                vaug[:, ki, h, :],
                                start=(ki == 0), stop=(ki == lastki))
                    rec = small.tile([128, HPC, 1], f32, tag="rec")
                    nc.vector.reciprocal(rec[:], pv[:, :, D:D + 1])
                    g = 4 * qi + qq
                    nc.vector.tensor_mul(
                        out=y_sb[:, :, g, :].rearrange(
                            "p s (e d) -> p s e d", e=2),
                        in0=pv[:, :, 0:D].rearrange(
                            "p (s e) d -> p s e d", s=2),
                        in1=rec[:].rearrange("p (s e) o -> p s e o", s=2)
                            .to_broadcast([128, 2, 2, D]))
                    for s in range(2):
                        ytr = ps_pv.tile([128, 128], bf16, tag="pv")
                        nc.tensor.transpose(ytr[:], y_sb[:, s, g, :], id_sb[:])
                        nc.vector.tensor_copy(yT_sb[:, s, ts(g, 128)], ytr[:])

                # ---- output projection, deferred one stripe so its PE/DVE
                #      work fills the ACT-bound phases of later stripes ----
                def outproj_group(st):
                    for g in range(4 * st, 4 * st + 4):
                        ot = op.tile([128, 1024], bf16, tag="ot")
                        for oi in range(2):
                            po = ps_qkv.tile([128, 512], f32, tag="qkv")
                            for s in range(2):
                                nc.tensor.matmul(
                                    po[:], yT_sb[:, s, ts(g, 128)],
                                    wpt_sb[:, s, ts(oi, 512)],
                                    start=(s == 0), stop=(s == 1))
                            nc.vector.tensor_copy(ot[:, ts(oi, 512)], po[:])
                        nc.sync.dma_start(out[ts(g, 128), :], ot[:])
                if ti >= 1:
                    outproj_group(ti - 1)
                if ti == NT512 - 1:
                    outproj_group(ti)

    nc.compile()
    return nc


def _get_compiled():
    global _COMPILED
    if _COMPILED is None:
        _COMPILED = _build()
    return _COMPILED


def _host_prep(x, W_attn, b_attn, W_proj, b_proj):
    xTb = [np.ascontiguousarray(x[b].T).astype(ml_dtypes.bfloat16)
           for b in range(B)]
    tri = (np.arange(128, dtype=np.int32)[None, :]
           >= np.arange(128, dtype=np.int32)[:, None])
    Sm = np.ascontiguousarray(
        np.tile(tri, (1, 4))).astype(ml_dtypes.bfloat16)
    in_maps = []
    for c in range(N_CORES):
        b, g = divmod(c, 4)
        ch = slice(CH * g, CH * (g + 1))
        Wq = W_attn[ch]
        Wk = W_attn[C:][ch]
        Wv = W_attn[2 * C:][ch]
        wt_c = np.ascontiguousarray(
            np.concatenate([Wq, Wk, Wv], axis=0).T).astype(ml_dtypes.bfloat16)
        bq = b_attn[ch]
        bk = b_attn[C:][ch]
        bv = b_attn[2 * C:][ch]
        bqk_c = np.ascontiguousarray(
            np.concatenate([bq, bk]).reshape(4, 128).T).astype(np.float32)
        bvb2_c = np.ascontiguousarray(
            np.tile(np.broadcast_to(bv[None, :], (128, CH)),
                    (1, 2))).astype(np.float32)
        wpt_c = np.ascontiguousarray(
            W_proj[:, ch].T).astype(ml_dtypes.bfloat16)
        in_maps.append({
            "xT": xTb[b],
            "wt": wt_c,
            "wpt": wpt_c,
            "bqk": bqk_c,
            "bvb2": bvb2_c,
            "Sm": Sm,
            "ident": np.eye(128, dtype=ml_dtypes.bfloat16),
        })
    return in_maps


def kernel(x, W_attn, b_attn, W_proj, b_proj):
    x = np.asarray(x, dtype=np.float32)
    W_attn = np.asarray(W_attn, dtype=np.float32)
    b_attn = np.asarray(b_attn, dtype=np.float32)
    W_proj = np.asarray(W_proj, dtype=np.float32)
    b_proj = np.asarray(b_proj, dtype=np.float32)

    nc = _get_compiled()
    in_maps = _host_prep(x, W_attn, b_attn, W_proj, b_proj)
    res = run_bass_kernel_spmd(nc, in_maps, core_ids=list(range(N_CORES)))

    out = np.empty((B, T, C), dtype=np.float32)
    for b in range(B):
        acc = np.asarray(res.results[4 * b]["out_partial"]).astype(np.float32)
        for g in range(1, 4):
            acc += np.asarray(
                res.results[4 * b + g]["out_partial"]).astype(np.float32)
        out[b] = acc + b_proj
    return out


# revision 36
# speedup vs baseline: 1.3006x; 1.0444x over previous
"""Causal self-attention on 8 NeuronCores (Bass/Tile).

Sharding: tensor-parallel over heads x data-parallel over batch.
  core c -> batch b = c//4, heads 4g..4g+3 where g = c%4.

v2 design (vs v1 baseline):
- all activations/weights bf16 (halves DMA + DVE traffic; matmuls run at
  1 cyc/row either way).
- scores run as fp8e4 DoubleRow matmuls at 0.5 cyc/row: contraction d=64
  is padded to a 2-tile DoubleRow pair whose second slot is all-zero
  (q/k slot-1 memset once), so out = k.T q exactly. The 1/sqrt(d) scale is
  applied by the Exp activation's scale operand, keeping q/k at their
  natural magnitude for fp8.
- p@v runs transposed-from-v1: out [q=128, d+1] with p as stationary, so
  cost is 65 rows per (q-tile, k-tile) instead of 512 (2x less PE).
- y [tok, ch] is transposed to [ch, tok] for the output projection by PE
  transposes (128x128 bf16, 53ns each) + DVE psum evacuation.
- exp is batched over ki-pairs ([128, 1024] per activation), bias adds are
  batched over fj-pairs / the 4 heads, the causal triangle mask is one
  [128, 4, 128] DVE multiply per diagonal block.
- output partials are written bf16 and summed on host (fp32) with b_proj.
"""

import os
import sys

for _p in ("/opt/trn_rl_repo", "/opt/pypackages"):
    if os.path.isdir(_p) and _p not in sys.path:
        sys.path.append(_p)

import numpy as np
import ml_dtypes

import concourse.bass as bass
import concourse.tile as tile
import concourse.mybir as mybir
from concourse import bacc
from concourse.bass_utils import run_bass_kernel_spmd

B, T, C = 2, 2048, 1024
H = 16            # total heads
D = 64            # head dim
HPC = 4           # heads per core
CH = HPC * D      # 256 channels per core
N_CORES = 8

f32 = mybir.dt.float32
bf16 = mybir.dt.bfloat16
fp8 = mybir.dt.float8e4
ts = bass.ts
ds = bass.ds
DR = mybir.MatmulPerfMode.DoubleRow
Exp = mybir.ActivationFunctionType.Exp

FP8_SCORES = os.environ.get("FP8_SCORES", "1") == "1"

_COMPILED = None


def _build():
    nc = bacc.Bacc("TRN2", target_bir_lowering=False, debug=False,
                   num_devices=N_CORES)

    xT = nc.dram_tensor("xT", [C, T], bf16, kind="ExternalInput").ap()
    wt = nc.dram_tensor("wt", [C, 3 * CH], bf16, kind="ExternalInput").ap()
    wpt = nc.dram_tensor("wpt", [CH, C], bf16, kind="ExternalInput").ap()
    bqk = nc.dram_tensor("bqk", [128, 4], f32, kind="ExternalInput").ap()
    bvb2 = nc.dram_tensor("bvb2", [128, 512], f32, kind="ExternalInput").ap()
    Sm = nc.dram_tensor("Sm", [128, 512], bf16, kind="ExternalInput").ap()
    ident = nc.dram_tensor("ident", [128, 128], bf16, kind="ExternalInput").ap()
    out = nc.dram_tensor("out_partial", [T, C], bf16, kind="ExternalOutput").ap()

    NT512 = T // 512          # 4   512-token stripes
    NT128 = T // 128          # 16  128-token tiles
    NC128 = C // 128          # 8   contraction tiles

    kdt = fp8 if FP8_SCORES else bf16

    xT_r = xT.rearrange("(o p) t -> p o t", p=128)
    wt_r = wt.rearrange("(o p) f -> p o f", p=128)

    with tile.TileContext(nc) as tc:
        with tc.tile_pool(name="consts", bufs=1) as consts, \
             tc.tile_pool(name="qkv", bufs=1) as qkv, \
             tc.tile_pool(name="xp", bufs=4) as xp, \
             tc.tile_pool(name="pp", bufs=10) as pp, \
             tc.tile_pool(name="op", bufs=3) as op, \
             tc.tile_pool(name="small", bufs=4) as small, \
             tc.tile_pool(name="ps_qkv", bufs=2, space="PSUM") as ps_qkv, \
             tc.tile_pool(name="ps_big", bufs=2, space="PSUM") as ps_big, \
             tc.tile_pool(name="ps_pv", bufs=2, space="PSUM") as ps_pv:

            # ---- constants; chunked so stripe-0 chains start after the
            #      first half of wt/x is resident ----
            wt_sb = consts.tile([128, NC128, 3 * CH], bf16)
            xt0 = xp.tile([128, NC128, 512], bf16, tag="xt")
            nc.sync.dma_start(wt_sb[:, :4, :512], wt_r[:, :4, :512])
            nc.sync.dma_start(xt0[:, :4], xT_r[:, :4, ts(0, 512)])
            bqk_sb = consts.tile([128, 4], f32)
            nc.sync.dma_start(bqk_sb[:], bqk)
            nc.sync.dma_start(wt_sb[:, 4:, :512], wt_r[:, 4:, :512])
            nc.sync.dma_start(xt0[:, 4:], xT_r[:, 4:, ts(0, 512)])
            nc.sync.dma_start(wt_sb[:, :, 512:], wt_r[:, :, 512:])
            bvb_sb = consts.tile([128, 2, 4, 64], f32)
            nc.sync.dma_start(
                bvb_sb[:], bvb2.rearrange("p (m h d) -> p m h d", m=2, h=4))
            S4_sb = consts.tile([128, 4, 128], bf16)
            nc.sync.dma_start(S4_sb[:], Sm.rearrange("p (h c) -> p h c", h=4))
            id_sb = consts.tile([128, 128], bf16)
            nc.sync.dma_start(id_sb[:], ident)
            xts = [xt0]
            for ti in range(1, NT512):
                xt = xp.tile([128, NC128, 512], bf16, tag="xt")
                nc.sync.dma_start(xt[:], xT_r[:, :, ts(ti, 512)])
                xts.append(xt)

            # ---- persistent activations ----
            # q/k live in bf16 [part=2heads*64d, slab, t]; stripes >= 1 also
            # get an fp8 copy in DoubleRow layout [part, slot, slab, t] whose
            # slot 1 is all-zero, so the fp8 DoubleRow pair sums to k.T q.
            # Stripe 0 (short attention rows, no error averaging) keeps its
            # scores in bf16.
            # bf16 q/k only needed for stripe 0's scores when fp8 is on
            qkT = 512 if FP8_SCORES else T
            qT_bf = qkv.tile([128, 2, qkT], bf16)
            kT_bf = qkv.tile([128, 2, qkT], bf16)
            vaug = qkv.tile([128, NT128, HPC, D + 1], bf16)  # [tok, ti, h, d|1]
            y_sb = qkv.tile([128, 2, NT128, 128], bf16)      # [tok, slab, tg, ch']
            yT_sb = qkv.tile([128, 2, T], bf16)              # [ch', slab, tok]
            if FP8_SCORES:
                qT8 = qkv.tile([128, 2, 2, T], fp8)
                kT8 = qkv.tile([128, 2, 2, T], fp8)

            # PE warmup: keep the tensor engine continuously busy from t~0.5us
            # until the first real chains, so the p-state ramp completes and
            # never resets. Uses the pv psum ring, whose first real use is far
            # later, so the ring slot is free again in time.
            wu = consts.tile([128, 512], bf16)
            nc.vector.memset(wu[:], 0.0)
            wu_ps = ps_pv.tile([128, HPC, D + 1], f32, tag="pv")
            for _ in range(45):
                nc.tensor.matmul(wu_ps[:].rearrange("p h d -> p (h d)"),
                                 wu[:, :128], wu[:, :HPC * (D + 1)],
                                 start=True, stop=True)

            if FP8_SCORES:
                nc.gpsimd.memset(qT8[:, 1], 0.0)
                nc.gpsimd.memset(kT8[:, 1], 0.0)
            nc.vector.memset(vaug[:, :, :, D:D + 1], 1.0)

            wpt_sb = consts.tile([128, 2, C], bf16)
            nc.sync.dma_start(
                wpt_sb[:], wpt.rearrange("(s p) o -> p s o", p=128))

            def qkv_fj(ti, fj, src, slab):
                # stripes >= 1 evacuate straight to the fp8 DoubleRow copy;
                # stripe 0 (bf16 scores) evacuates to the bf16 tiles.
                xt = xts[ti]
                ps = ps_qkv.tile([128, 512], f32, tag="qkv")
                for ci in range(NC128):
                    nc.tensor.matmul(
                        ps[:], wt_sb[:, ci, ts(fj, 128)], xt[:, ci, :],
                        start=(ci == 0), stop=(ci == NC128 - 1))
                if FP8_SCORES and ti > 0:
                    dst8 = qT8 if src is qT_bf else kT8
                    dest = dst8[:, 0, slab, ts(ti, 512)]
                else:
                    dest = src[:, slab, ts(ti, 512)]
                nc.vector.tensor_add(
                    out=dest, in0=ps[:],
                    in1=bqk_sb[:, fj:fj + 1].to_broadcast([128, 512]))

            def qkv_v(ti):
                xt = xts[ti]
                for m in range(2):
                    pv2 = ps_qkv.tile([128, 2, 4, 64], f32, tag="qkv")
                    for tj in range(2):
                        for ci in range(NC128):
                            nc.tensor.matmul(
                                pv2[:, tj], xt[:, ci, ts(2 * m + tj, 128)],
                                wt_sb[:, ci, 512:512 + CH],
                                start=(ci == 0), stop=(ci == NC128 - 1))
                    nc.vector.tensor_add(
                        out=vaug[:, 4 * ti + 2 * m:4 * ti + 2 * m + 2, :, 0:D],
                        in0=pv2[:], in1=bvb_sb[:])

            def conv_fp8(ti, src, slab):
                # stripe-0 k also needs an fp8 copy for later stripes' scores;
                # it is off the critical path, so GpSimd does the cast.
                if not FP8_SCORES or ti > 0 or src is qT_bf:
                    return
                nc.gpsimd.tensor_copy(
                    kT8[:, 0, slab, ts(ti, 512)], src[:, slab, ts(ti, 512)])

            def qk_stripe(ti):
                # q slab 0 first: the next stripe's first scores gate on it
                for fj, src, slab in ((0, qT_bf, 0), (2, kT_bf, 0),
                                      (1, qT_bf, 1), (3, kT_bf, 1)):
                    qkv_fj(ti, fj, src, slab)
                    conv_fp8(ti, src, slab)

            def emit_pair(qi, pi, p_t, heads):
                """Scores + exp for pair pi of stripe qi, given heads only.
                Masks are NOT emitted here (need all 4 heads done)."""
                use8 = FP8_SCORES and qi > 0
                diag = pi >= 2 * qi
                j0 = 2 * pi - 4 * qi
                if diag:
                    w0 = 512 - 128 * j0
                    offs, q0s = (0, w0), (128 * j0, 128 * (j0 + 1))
                    wtot = w0 + (384 - 128 * j0)
                else:
                    offs, q0s, wtot = (0, 512), (0, 0), 1024
                for h in heads:
                    hp, hs = (h % 2) * D, h // 2
                    psc = ps_big.tile([128, 1024], f32, tag="big")
                    for half in range(2):
                        ki = 2 * pi + half
                        w = 512 - q0s[half]
                        if use8:
                            nc.tensor.matmul(
                                psc[:, ds(offs[half], w)],
                                kT8[hp:hp + D, :, hs, ts(ki, 128)],
                                qT8[hp:hp + D, :, hs,
                                    ds(512 * qi + q0s[half], w)],
                                start=True, stop=True, perf_mode=DR)
                        else:
                            nc.tensor.matmul(
                                psc[:, ds(offs[half], w)],
                                kT_bf[hp:hp + D, hs, ts(ki, 128)],
                                qT_bf[hp:hp + D, hs,
                                      ds(512 * qi + q0s[half], w)],
                                start=True, stop=True)
                    nc.scalar.activation(
                        p_t[:, h, 0:wtot], psc[:, 0:wtot], Exp, scale=0.125)
                return offs if diag else None

            def emit_mask(p_t, offs, hgs=(0, 1)):
                # split per head-pair so late chains unblock as soon as their
                # heads' exps land
                for hg in hgs:
                    for half in range(2):
                        nc.vector.tensor_mul(
                            out=p_t[:, 2 * hg:2 * hg + 2, ds(offs[half], 128)],
                            in0=p_t[:, 2 * hg:2 * hg + 2, ds(offs[half], 128)],
                            in1=S4_sb[:, 2 * hg:2 * hg + 2, :])

            # stripe 0 fast path: interleave its (bf16) pair-0 scores with the
            # qkv chains so the ACT pipeline starts as early as possible.
            p0_t = pp.tile([128, HPC, 1024], bf16, tag="p")
            qkv_fj(0, 0, qT_bf, 0)
            qkv_fj(0, 2, kT_bf, 0)
            conv_fp8(0, kT_bf, 0)
            with tc.high_priority(offset=8000):
                offs0 = emit_pair(0, 0, p0_t, (0, 1))
                emit_mask(p0_t, offs0, hgs=(0,))
            qkv_fj(0, 1, qT_bf, 1)
            qkv_fj(0, 3, kT_bf, 1)
            conv_fp8(0, kT_bf, 1)
            with tc.high_priority(offset=8000):
                emit_pair(0, 0, p0_t, (2, 3))
                emit_mask(p0_t, offs0, hgs=(1,))

            for ti in range(NT512):
                # ---------- attention stripe qi = ti ----------
                # p tiles use a packed column layout [128, h, 1024]: for
                # off-diagonal ki-pairs, cols [512*half : 512*half+512]; for
                # diagonal pairs the two ragged blocks [q0:512] are packed
                # back-to-back at [0:w0] and [w0:w0+w1] so exp is one
                # activation per (pair, head).
                qi = ti
                nk = 4 * qi + 4
                npair = nk // 2

                def pcol(ki, qq):
                    # packed p-tile column of query subtile qq for key tile ki
                    j = ki - 4 * qi
                    if j < 0:
                        return 512 * (ki % 2) + 128 * qq
                    half = j % 2
                    off = 0 if half == 0 else 512 - 128 * (j - 1)
                    return off + 128 * (qq - j)

                # scores/exp outrank all other ready work: they are cheap on
                # PE but gate the ACT stream, which is the secondary
                # bottleneck.
                p_tiles = []
                with tc.high_priority(offset=8000):
                    for pi in range(npair):
                        if qi == 0 and pi == 0:
                            p_tiles.append(p0_t)
                            continue
                        p_t = pp.tile([128, HPC, 1024], bf16, tag="p")
                        p_tiles.append(p_t)
                        offs = emit_pair(qi, pi, p_t, (0, 1))
                        if offs is not None:
                            emit_mask(p_t, offs, hgs=(0,))
                        emit_pair(qi, pi, p_t, (2, 3))
                        if offs is not None:
                            emit_mask(p_t, offs, hgs=(1,))

                # q/k for the next stripe gate the next ACT phase, so they
                # go ahead of this stripe's pv/outproj filler work; the next
                # v chains are pure filler and go after the pv chains.
                if ti + 1 < NT512:
                    qk_stripe(ti + 1)
                qkv_v(ti)

                for qq in range(4):
                    pv = ps_pv.tile([128, HPC, D + 1], f32, tag="pv")
                    lastki = 4 * qi + qq
                    for h in range(HPC):
                        for ki in range(lastki + 1):
                            nc.tensor.matmul(
                                pv[:, h],
                                p_tiles[ki // 2][:, h, ds(pcol(ki, qq), 128)],
                                vaug[:, ki, h, :],
                                start=(ki == 0), stop=(ki == lastki))
                    rec = small.tile([128, HPC, 1], f32, tag="rec")
                    nc.vector.reciprocal(rec[:], pv[:, :, D:D + 1])
                    g = 4 * qi + qq
                    nc.vector.tensor_mul(
                        out=y_sb[:, :, g, :].rearrange(
                            "p s (e d) -> p s e d", e=2),
                        in0=pv[:, :, 0:D].rearrange(
                            "p (s e) d -> p s e d", s=2),
                        in1=rec[:].rearrange("p (s e) o -> p s e o", s=2)
                            .to_broadcast([128, 2, 2, D]))
                    for s in range(2):
                        ytr = ps_pv.tile([128, 128], bf16, tag="pv")
                        nc.tensor.transpose(ytr[:], y_sb[:, s, g, :], id_sb[:])
                        nc.vector.tensor_copy(yT_sb[:, s, ts(g, 128)], ytr[:])

                # ---- output projection, deferred one stripe so its PE/DVE
                #      work fills the ACT-bound phases of later stripes ----
                def outproj_group(st):
                    last = st == NT512 - 1
                    for g in range(4 * st, 4 * st + 4):
                        ot = op.tile([128, 1024], bf16, tag="ot")
                        for oi in range(2):
                            po = ps_qkv.tile([128, 512], f32, tag="qkv")
                            for s in range(2):
                                nc.tensor.matmul(
                                    po[:], yT_sb[:, s, ts(g, 128)],
                                    wpt_sb[:, s, ts(oi, 512)],
                                    start=(s == 0), stop=(s == 1))
                            if last and oi == 1:
                                # ACT is idle in the kernel tail; splitting the
                                # evacuation across engines shortens the final
                                # cascade
                                nc.scalar.activation(
                                    ot[:, ts(oi, 512)], po[:],
                                    mybir.ActivationFunctionType.Copy)
                            else:
                                nc.vector.tensor_copy(ot[:, ts(oi, 512)], po[:])
                        nc.sync.dma_start(out[ts(g, 128), :], ot[:])
                if ti >= 1:
                    outproj_group(ti - 1)
                if ti == NT512 - 1:
                    outproj_group(ti)

    nc.compile()
    return nc


def _get_compiled():
    global _COMPILED
    if _COMPILED is None:
        _COMPILED = _build()
    return _COMPILED


def _host_prep(x, W_attn, b_attn, W_proj, b_proj):
    xTb = [np.ascontiguousarray(x[b].T).astype(ml_dtypes.bfloat16)
           for b in range(B)]
    tri = (np.arange(128, dtype=np.int32)[None, :]
           >= np.arange(128, dtype=np.int32)[:, None])
    Sm = np.ascontiguousarray(
        np.tile(tri, (1, 4))).astype(ml_dtypes.bfloat16)
    in_maps = []
    for c in range(N_CORES):
        b, g = divmod(c, 4)
        ch = slice(CH * g, CH * (g + 1))
        Wq = W_attn[ch]
        Wk = W_attn[C:][ch]
        Wv = W_attn[2 * C:][ch]
        wt_c = np.ascontiguousarray(
            np.concatenate([Wq, Wk, Wv], axis=0).T).astype(ml_dtypes.bfloat16)
        bq = b_attn[ch]
        bk = b_attn[C:][ch]
        bv = b_attn[2 * C:][ch]
        bqk_c = np.ascontiguousarray(
            np.concatenate([bq, bk]).reshape(4, 128).T).astype(np.float32)
        bvb2_c = np.ascontiguousarray(
            np.tile(np.broadcast_to(bv[None, :], (128, CH)),
                    (1, 2))).astype(np.float32)
        wpt_c = np.ascontiguousarray(
            W_proj[:, ch].T).astype(ml_dtypes.bfloat16)
        in_maps.append({
            "xT": xTb[b],
            "wt": wt_c,
            "wpt": wpt_c,
            "bqk": bqk_c,
            "bvb2": bvb2_c,
            "Sm": Sm,
            "ident": np.eye(128, dtype=ml_dtypes.bfloat16),
        })
    return in_maps


def kernel(x, W_attn, b_attn, W_proj, b_proj):
    x = np.asarray(x, dtype=np.float32)
    W_attn = np.asarray(W_attn, dtype=np.float32)
    b_attn = np.asarray(b_attn, dtype=np.float32)
    W_proj = np.asarray(W_proj, dtype=np.float32)
    b_proj = np.asarray(b_proj, dtype=np.float32)

    nc = _get_compiled()
    in_maps = _host_prep(x, W_attn, b_attn, W_proj, b_proj)
    res = run_bass_kernel_spmd(nc, in_maps, core_ids=list(range(N_CORES)))

    out = np.empty((B, T, C), dtype=np.float32)
    for b in range(B):
        acc = np.asarray(res.results[4 * b]["out_partial"]).astype(np.float32)
        for g in range(1, 4):
            acc += np.asarray(
                res.results[4 * b + g]["out_partial"]).astype(np.float32)
        out[b] = acc + b_proj
    return out


# revision 44
# speedup vs baseline: 1.3083x; 1.0059x over previous
"""Causal self-attention on 8 NeuronCores (Bass/Tile).

Sharding: tensor-parallel over heads x data-parallel over batch.
  core c -> batch b = c//4, heads 4g..4g+3 where g = c%4.

Design (vs the 146us v1 baseline; modeled 111.7us, ~1.31x):
- all activations/weights bf16 (halves DMA + DVE traffic; matmuls run at
  1 cyc/row either way).
- scores for stripes >= 1 run as fp8e4 DoubleRow matmuls at 0.5 cyc/row:
  contraction d=64 is padded to a 2-tile DoubleRow pair whose second slot
  is all-zero (q/k slot-1 memset once), so out = k.T q exactly. Stripe 0
  keeps bf16 scores: its short attention rows get no error averaging, so
  fp8 noise there dominates the overall error. The 1/sqrt(d) scale is
  applied by the Exp activation's scale operand, keeping q/k at their
  natural fp8-friendly magnitude. DVE evacuates q/k straight into the fp8
  DoubleRow layout for stripes >= 1; GpSimd casts stripe-0's k.
- p@v runs transposed-from-v1: out [q=128, d+1] with p as stationary, so
  cost is 65 rows per (q-tile, k-tile) instead of 512 (2x less PE).
- y [tok, ch] is transposed to [ch, tok] for the output projection by PE
  transposes (128x128 bf16, 53ns each) + DVE psum evacuation.
- exp is batched over ki-pairs with diagonal blocks packed back-to-back
  (one [128, <=1024] activation per pair/head); bias adds are batched over
  fj/head dims; the causal triangle mask is a [128, 2, 128] DVE multiply
  per diagonal block and head-pair.
- scheduling: PE warmup abuts the first chains so the p-state ramp never
  resets; scores/exps get a large priority boost (cheap on PE, gate the
  74us ACT stream); next-stripe q/k chains precede this stripe's pv/v
  work; output projection is deferred one stripe so it fills the ACT-bound
  tail phases; stripe-0 scores interleave with the qkv chains.
- output partials are written bf16 and summed on host (fp32) with b_proj.
"""

import os
import sys

for _p in ("/opt/trn_rl_repo", "/opt/pypackages"):
    if os.path.isdir(_p) and _p not in sys.path:
        sys.path.append(_p)

import numpy as np
import ml_dtypes

import concourse.bass as bass
import concourse.tile as tile
import concourse.mybir as mybir
from concourse import bacc
from concourse.bass_utils import run_bass_kernel_spmd

B, T, C = 2, 2048, 1024
H = 16            # total heads
D = 64            # head dim
HPC = 4           # heads per core
CH = HPC * D      # 256 channels per core
N_CORES = 8

f32 = mybir.dt.float32
bf16 = mybir.dt.bfloat16
fp8 = mybir.dt.float8e4
ts = bass.ts
ds = bass.ds
DR = mybir.MatmulPerfMode.DoubleRow
Exp = mybir.ActivationFunctionType.Exp

FP8_SCORES = os.environ.get("FP8_SCORES", "1") == "1"

_COMPILED = None


def _build():
    nc = bacc.Bacc("TRN2", target_bir_lowering=False, debug=False,
                   num_devices=N_CORES)

    xT = nc.dram_tensor("xT", [C, T], bf16, kind="ExternalInput").ap()
    wt = nc.dram_tensor("wt", [C, 3 * CH], bf16, kind="ExternalInput").ap()
    wpt = nc.dram_tensor("wpt", [CH, C], bf16, kind="ExternalInput").ap()
    bqk = nc.dram_tensor("bqk", [128, 4], f32, kind="ExternalInput").ap()
    bvb2 = nc.dram_tensor("bvb2", [128, 512], f32, kind="ExternalInput").ap()
    Sm = nc.dram_tensor("Sm", [128, 512], bf16, kind="ExternalInput").ap()
    ident = nc.dram_tensor("ident", [128, 128], bf16, kind="ExternalInput").ap()
    out = nc.dram_tensor("out_partial", [T, C], bf16, kind="ExternalOutput").ap()

    NT512 = T // 512          # 4   512-token stripes
    NT128 = T // 128          # 16  128-token tiles
    NC128 = C // 128          # 8   contraction tiles

    kdt = fp8 if FP8_SCORES else bf16

    xT_r = xT.rearrange("(o p) t -> p o t", p=128)
    wt_r = wt.rearrange("(o p) f -> p o f", p=128)

    with tile.TileContext(nc) as tc:
        with tc.tile_pool(name="consts", bufs=1) as consts, \
             tc.tile_pool(name="qkv", bufs=1) as qkv, \
             tc.tile_pool(name="xp", bufs=4) as xp, \
             tc.tile_pool(name="pp", bufs=11) as pp, \
             tc.tile_pool(name="op", bufs=4) as op, \
             tc.tile_pool(name="small", bufs=4) as small, \
             tc.tile_pool(name="ps_qkv", bufs=2, space="PSUM") as ps_qkv, \
             tc.tile_pool(name="ps_big", bufs=2, space="PSUM") as ps_big, \
             tc.tile_pool(name="ps_pv", bufs=2, space="PSUM") as ps_pv:

            # ---- constants; chunked so stripe-0 chains start after the
            #      first half of wt/x is resident ----
            wt_sb = consts.tile([128, NC128, 3 * CH], bf16)
            xt0 = xp.tile([128, NC128, 512], bf16, tag="xt")
            nc.sync.dma_start(wt_sb[:, :4, :512], wt_r[:, :4, :512])
            nc.sync.dma_start(xt0[:, :4], xT_r[:, :4, ts(0, 512)])
            bqk_sb = consts.tile([128, 4], f32)
            nc.sync.dma_start(bqk_sb[:], bqk)
            nc.sync.dma_start(wt_sb[:, 4:, :512], wt_r[:, 4:, :512])
            nc.sync.dma_start(xt0[:, 4:], xT_r[:, 4:, ts(0, 512)])
            nc.sync.dma_start(wt_sb[:, :, 512:], wt_r[:, :, 512:])
            bvb_sb = consts.tile([128, 2, 4, 64], f32)
            nc.sync.dma_start(
                bvb_sb[:], bvb2.rearrange("p (m h d) -> p m h d", m=2, h=4))
            S4_sb = consts.tile([128, 4, 128], bf16)
            nc.sync.dma_start(S4_sb[:], Sm.rearrange("p (h c) -> p h c", h=4))
            id_sb = consts.tile([128, 128], bf16)
            nc.sync.dma_start(id_sb[:], ident)
            xts = [xt0]
            for ti in range(1, NT512):
                xt = xp.tile([128, NC128, 512], bf16, tag="xt")
                nc.sync.dma_start(xt[:], xT_r[:, :, ts(ti, 512)])
                xts.append(xt)

            # ---- persistent activations ----
            # q/k live in bf16 [part=2heads*64d, slab, t]; stripes >= 1 also
            # get an fp8 copy in DoubleRow layout [part, slot, slab, t] whose
            # slot 1 is all-zero, so the fp8 DoubleRow pair sums to k.T q.
            # Stripe 0 (short attention rows, no error averaging) keeps its
            # scores in bf16.
            # bf16 q/k only needed for stripe 0's scores when fp8 is on
            qkT = 512 if FP8_SCORES else T
            qT_bf = qkv.tile([128, 2, qkT], bf16)
            kT_bf = qkv.tile([128, 2, qkT], bf16)
            vaug = qkv.tile([128, NT128, HPC, D + 1], bf16)  # [tok, ti, h, d|1]
            y_sb = qkv.tile([128, 2, NT128, 128], bf16)      # [tok, slab, tg, ch']
            yT_sb = qkv.tile([128, 2, T], bf16)              # [ch', slab, tok]
            if FP8_SCORES:
                qT8 = qkv.tile([128, 2, 2, T], fp8)
                kT8 = qkv.tile([128, 2, 2, T], fp8)

            # PE warmup: keep the tensor engine continuously busy from t~0.5us
            # until the first real chains, so the p-state ramp completes and
            # never resets. Uses the pv psum ring, whose first real use is far
            # later, so the ring slot is free again in time.
            wu = consts.tile([128, 512], bf16)
            nc.vector.memset(wu[:], 0.0)
            wu_ps = ps_pv.tile([128, HPC, D + 1], f32, tag="pv")
            for _ in range(45):
                nc.tensor.matmul(wu_ps[:].rearrange("p h d -> p (h d)"),
                                 wu[:, :128], wu[:, :HPC * (D + 1)],
                                 start=True, stop=True)

            if FP8_SCORES:
                nc.gpsimd.memset(qT8[:, 1], 0.0)
                nc.gpsimd.memset(kT8[:, 1], 0.0)
            nc.vector.memset(vaug[:, :, :, D:D + 1], 1.0)

            wpt_sb = consts.tile([128, 2, C], bf16)
            nc.sync.dma_start(
                wpt_sb[:], wpt.rearrange("(s p) o -> p s o", p=128))

            def qkv_fj(ti, fj, src, slab):
                # stripes >= 1 evacuate straight to the fp8 DoubleRow copy;
                # stripe 0 (bf16 scores) evacuates to the bf16 tiles.
                xt = xts[ti]
                ps = ps_qkv.tile([128, 512], f32, tag="qkv")
                for ci in range(NC128):
                    nc.tensor.matmul(
                        ps[:], wt_sb[:, ci, ts(fj, 128)], xt[:, ci, :],
                        start=(ci == 0), stop=(ci == NC128 - 1))
                if FP8_SCORES and ti > 0:
                    dst8 = qT8 if src is qT_bf else kT8
                    dest = dst8[:, 0, slab, ts(ti, 512)]
                else:
                    dest = src[:, slab, ts(ti, 512)]
                nc.vector.tensor_add(
                    out=dest, in0=ps[:],
                    in1=bqk_sb[:, fj:fj + 1].to_broadcast([128, 512]))

            def qkv_v(ti):
                xt = xts[ti]
                for m in range(2):
                    pv2 = ps_qkv.tile([128, 2, 4, 64], f32, tag="qkv")
                    for tj in range(2):
                        for ci in range(NC128):
                            nc.tensor.matmul(
                                pv2[:, tj], xt[:, ci, ts(2 * m + tj, 128)],
                                wt_sb[:, ci, 512:512 + CH],
                                start=(ci == 0), stop=(ci == NC128 - 1))
                    nc.vector.tensor_add(
                        out=vaug[:, 4 * ti + 2 * m:4 * ti + 2 * m + 2, :, 0:D],
                        in0=pv2[:], in1=bvb_sb[:])

            def conv_fp8(ti, src, slab):
                # stripe-0 k also needs an fp8 copy for later stripes' scores;
                # it is off the critical path, so GpSimd does the cast.
                if not FP8_SCORES or ti > 0 or src is qT_bf:
                    return
                nc.gpsimd.tensor_copy(
                    kT8[:, 0, slab, ts(ti, 512)], src[:, slab, ts(ti, 512)])

            def qk_stripe(ti):
                # q slab 0 first: the next stripe's first scores gate on it
                for fj, src, slab in ((0, qT_bf, 0), (2, kT_bf, 0),
                                      (1, qT_bf, 1), (3, kT_bf, 1)):
                    qkv_fj(ti, fj, src, slab)
                    conv_fp8(ti, src, slab)

            def emit_pair(qi, pi, p_t, heads):
                """Scores + exp for pair pi of stripe qi, given heads only.
                Masks are NOT emitted here (need all 4 heads done)."""
                use8 = FP8_SCORES and qi > 0
                diag = pi >= 2 * qi
                j0 = 2 * pi - 4 * qi
                if diag:
                    w0 = 512 - 128 * j0
                    offs, q0s = (0, w0), (128 * j0, 128 * (j0 + 1))
                    wtot = w0 + (384 - 128 * j0)
                else:
                    offs, q0s, wtot = (0, 512), (0, 0), 1024
                for h in heads:
                    hp, hs = (h % 2) * D, h // 2
                    psc = ps_big.tile([128, 1024], f32, tag="big")
                    for half in range(2):
                        ki = 2 * pi + half
                        w = 512 - q0s[half]
                        if use8:
                            nc.tensor.matmul(
                                psc[:, ds(offs[half], w)],
                                kT8[hp:hp + D, :, hs, ts(ki, 128)],
                                qT8[hp:hp + D, :, hs,
                                    ds(512 * qi + q0s[half], w)],
                                start=True, stop=True, perf_mode=DR)
                        else:
                            nc.tensor.matmul(
                                psc[:, ds(offs[half], w)],
                                kT_bf[hp:hp + D, hs, ts(ki, 128)],
                                qT_bf[hp:hp + D, hs,
                                      ds(512 * qi + q0s[half], w)],
                                start=True, stop=True)
                    nc.scalar.activation(
                        p_t[:, h, 0:wtot], psc[:, 0:wtot], Exp, scale=0.125)
                return offs if diag else None

            def emit_mask(p_t, offs, hgs=(0, 1)):
                # split per head-pair so late chains unblock as soon as their
                # heads' exps land
                for hg in hgs:
                    for half in range(2):
                        nc.vector.tensor_mul(
                            out=p_t[:, 2 * hg:2 * hg + 2, ds(offs[half], 128)],
                            in0=p_t[:, 2 * hg:2 * hg + 2, ds(offs[half], 128)],
                            in1=S4_sb[:, 2 * hg:2 * hg + 2, :])

            # stripe 0 fast path: interleave its (bf16) pair-0 scores with the
            # qkv chains so the ACT pipeline starts as early as possible.
            p0_t = pp.tile([128, HPC, 1024], bf16, tag="p")
            qkv_fj(0, 0, qT_bf, 0)
            qkv_fj(0, 2, kT_bf, 0)
            conv_fp8(0, kT_bf, 0)
            with tc.high_priority(offset=8000):
                offs0 = emit_pair(0, 0, p0_t, (0, 1))
                emit_mask(p0_t, offs0, hgs=(0,))
            qkv_fj(0, 1, qT_bf, 1)
            qkv_fj(0, 3, kT_bf, 1)
            conv_fp8(0, kT_bf, 1)
            with tc.high_priority(offset=8000):
                emit_pair(0, 0, p0_t, (2, 3))
                emit_mask(p0_t, offs0, hgs=(1,))

            for ti in range(NT512):
                # ---------- attention stripe qi = ti ----------
                # p tiles use a packed column layout [128, h, 1024]: for
                # off-diagonal ki-pairs, cols [512*half : 512*half+512]; for
                # diagonal pairs the two ragged blocks [q0:512] are packed
                # back-to-back at [0:w0] and [w0:w0+w1] so exp is one
                # activation per (pair, head).
                qi = ti
                nk = 4 * qi + 4
                npair = nk // 2

                def pcol(ki, qq):
                    # packed p-tile column of query subtile qq for key tile ki
                    j = ki - 4 * qi
                    if j < 0:
                        return 512 * (ki % 2) + 128 * qq
                    half = j % 2
                    off = 0 if half == 0 else 512 - 128 * (j - 1)
                    return off + 128 * (qq - j)

                # scores/exp outrank all other ready work: they are cheap on
                # PE but gate the ACT stream, which is the secondary
                # bottleneck.
                p_tiles = []
                with tc.high_priority(offset=8000):
                    for pi in range(npair):
                        if qi == 0 and pi == 0:
                            p_tiles.append(p0_t)
                            continue
                        p_t = pp.tile([128, HPC, 1024], bf16, tag="p")
                        p_tiles.append(p_t)
                        offs = emit_pair(qi, pi, p_t, (0, 1))
                        if offs is not None:
                            emit_mask(p_t, offs, hgs=(0,))
                        emit_pair(qi, pi, p_t, (2, 3))
                        if offs is not None:
                            emit_mask(p_t, offs, hgs=(1,))

                # q/k for the next stripe gate the next ACT phase, so they
                # go ahead of this stripe's pv/outproj filler work; the next
                # v chains are pure filler and go after the pv chains.
                if ti + 1 < NT512:
                    qk_stripe(ti + 1)
                qkv_v(ti)

                for qq in range(4):
                    pv = ps_pv.tile([128, HPC, D + 1], f32, tag="pv")
                    lastki = 4 * qi + qq
                    for h in range(HPC):
                        for ki in range(lastki + 1):
                            nc.tensor.matmul(
                                pv[:, h],
                                p_tiles[ki // 2][:, h, ds(pcol(ki, qq), 128)],
                                vaug[:, ki, h, :],
                                start=(ki == 0), stop=(ki == lastki))
                    rec = small.tile([128, HPC, 1], f32, tag="rec")
                    nc.vector.reciprocal(rec[:], pv[:, :, D:D + 1])
                    g = 4 * qi + qq
                    nc.vector.tensor_mul(
                        out=y_sb[:, :, g, :].rearrange(
                            "p s (e d) -> p s e d", e=2),
                        in0=pv[:, :, 0:D].rearrange(
                            "p (s e) d -> p s e d", s=2),
                        in1=rec[:].rearrange("p (s e) o -> p s e o", s=2)
                            .to_broadcast([128, 2, 2, D]))
                    for s in range(2):
                        ytr = ps_pv.tile([128, 128], bf16, tag="pv")
                        nc.tensor.transpose(ytr[:], y_sb[:, s, g, :], id_sb[:])
                        nc.vector.tensor_copy(yT_sb[:, s, ts(g, 128)], ytr[:])

                # ---- output projection, deferred one stripe so its PE/DVE
                #      work fills the ACT-bound phases of later stripes ----
                def outproj_group(st):
                    last = st == NT512 - 1
                    for g in range(4 * st, 4 * st + 4):
                        ot = op.tile([128, 1024], bf16, tag="ot")
                        for oi in range(2):
                            po = ps_qkv.tile([128, 512], f32, tag="qkv")
                            for s in range(2):
                                nc.tensor.matmul(
                                    po[:], yT_sb[:, s, ts(g, 128)],
                                    wpt_sb[:, s, ts(oi, 512)],
                                    start=(s == 0), stop=(s == 1))
                            if last and oi == 1:
                                # ACT is idle in the kernel tail; splitting the
                                # evacuation across engines shortens the final
                                # cascade
                                nc.scalar.activation(
                                    ot[:, ts(oi, 512)], po[:],
                                    mybir.ActivationFunctionType.Copy)
                            else:
                                nc.vector.tensor_copy(ot[:, ts(oi, 512)], po[:])
                            if last:
                                nc.sync.dma_start(
                                    out[ts(g, 128), ts(oi, 512)],
                                    ot[:, ts(oi, 512)])
                        if not last:
                            nc.sync.dma_start(out[ts(g, 128), :], ot[:])
                if ti >= 1:
                    outproj_group(ti - 1)
                if ti == NT512 - 1:
                    outproj_group(ti)

    nc.compile()
    return nc


def _get_compiled():
    global _COMPILED
    if _COMPILED is None:
        _COMPILED = _build()
    return _COMPILED


def _host_prep(x, W_attn, b_attn, W_proj, b_proj):
    xTb = [np.ascontiguousarray(x[b].T).astype(ml_dtypes.bfloat16)
           for b in range(B)]
    tri = (np.arange(128, dtype=np.int32)[None, :]
           >= np.arange(128, dtype=np.int32)[:, None])
    Sm = np.ascontiguousarray(
        np.tile(tri, (1, 4))).astype(ml_dtypes.bfloat16)
    in_maps = []
    for c in range(N_CORES):
        b, g = divmod(c, 4)
        ch = slice(CH * g, CH * (g + 1))
        Wq = W_attn[ch]
        Wk = W_attn[C:][ch]
        Wv = W_attn[2 * C:][ch]
        wt_c = np.ascontiguousarray(
            np.concatenate([Wq, Wk, Wv], axis=0).T).astype(ml_dtypes.bfloat16)
        bq = b_attn[ch]
        bk = b_attn[C:][ch]
        bv = b_attn[2 * C:][ch]
        bqk_c = np.ascontiguousarray(
            np.concatenate([bq, bk]).reshape(4, 128).T).astype(np.float32)
        bvb2_c = np.ascontiguousarray(
            np.tile(np.broadcast_to(bv[None, :], (128, CH)),
                    (1, 2))).astype(np.float32)
        wpt_c = np.ascontiguousarray(
            W_proj[:, ch].T).astype(ml_dtypes.bfloat16)
        in_maps.append({
            "xT": xTb[b],
            "wt": wt_c,
            "wpt": wpt_c,
            "bqk": bqk_c,
            "bvb2": bvb2_c,
            "Sm": Sm,
            "ident": np.eye(128, dtype=ml_dtypes.bfloat16),
        })
    return in_maps


def kernel(x, W_attn, b_attn, W_proj, b_proj):
    x = np.asarray(x, dtype=np.float32)
    W_attn = np.asarray(W_attn, dtype=np.float32)
    b_attn = np.asarray(b_attn, dtype=np.float32)
    W_proj = np.asarray(W_proj, dtype=np.float32)
    b_proj = np.asarray(b_proj, dtype=np.float32)

    nc = _get_compiled()
    in_maps = _host_prep(x, W_attn, b_attn, W_proj, b_proj)
    res = run_bass_kernel_spmd(nc, in_maps, core_ids=list(range(N_CORES)))

    out = np.empty((B, T, C), dtype=np.float32)
    for b in range(B):
        acc = np.asarray(res.results[4 * b]["out_partial"]).astype(np.float32)
        for g in range(1, 4):
            acc += np.asarray(
                res.results[4 * b + g]["out_partial"]).astype(np.float32)
        out[b] = acc + b_proj
    return out


# revision 51
# speedup vs baseline: 1.3671x; 1.0449x over previous
"""Causal self-attention on 8 NeuronCores (Bass/Tile).

Sharding: tensor-parallel over heads x data-parallel over batch.
  core c -> batch b = c//4, heads 4g..4g+3 where g = c%4.

Design (vs the 146us v1 baseline; modeled 111.7us, ~1.31x):
- all activations/weights bf16 (halves DMA + DVE traffic; matmuls run at
  1 cyc/row either way).
- scores for stripes >= 1 run as fp8e4 DoubleRow matmuls at 0.5 cyc/row:
  contraction d=64 is padded to a 2-tile DoubleRow pair whose second slot
  is all-zero (q/k slot-1 memset once), so out = k.T q exactly. Stripe 0
  keeps bf16 scores: its short attention rows get no error averaging, so
  fp8 noise there dominates the overall error. The 1/sqrt(d) scale is
  applied by the Exp activation's scale operand, keeping q/k at their
  natural fp8-friendly magnitude. DVE evacuates q/k straight into the fp8
  DoubleRow layout for stripes >= 1; GpSimd casts stripe-0's k.
- p@v runs transposed-from-v1: out [q=128, d+1] with p as stationary, so
  cost is 65 rows per (q-tile, k-tile) instead of 512 (2x less PE).
- y [tok, ch] is transposed to [ch, tok] for the output projection by PE
  transposes (128x128 bf16, 53ns each) + DVE psum evacuation.
- exp is batched over ki-pairs with diagonal blocks packed back-to-back
  (one [128, <=1024] activation per pair/head); bias adds are batched over
  fj/head dims; the causal triangle mask is a [128, 2, 128] DVE multiply
  per diagonal block and head-pair.
- scheduling: PE warmup abuts the first chains so the p-state ramp never
  resets; scores/exps get a large priority boost (cheap on PE, gate the
  74us ACT stream); next-stripe q/k chains precede this stripe's pv/v
  work; output projection is deferred one stripe so it fills the ACT-bound
  tail phases; stripe-0 scores interleave with the qkv chains.
- output partials are written bf16 and summed on host (fp32) with b_proj.
"""

import os
import sys

for _p in ("/opt/trn_rl_repo", "/opt/pypackages"):
    if os.path.isdir(_p) and _p not in sys.path:
        sys.path.append(_p)

import numpy as np
import ml_dtypes

import concourse.bass as bass
import concourse.tile as tile
import concourse.mybir as mybir
from concourse import bacc
from concourse.bass_utils import run_bass_kernel_spmd

B, T, C = 2, 2048, 1024
H = 16            # total heads
D = 64            # head dim
HPC = 4           # heads per core
CH = HPC * D      # 256 channels per core
N_CORES = 8

f32 = mybir.dt.float32
bf16 = mybir.dt.bfloat16
fp8 = mybir.dt.float8e4
ts = bass.ts
ds = bass.ds
DR = mybir.MatmulPerfMode.DoubleRow
Exp = mybir.ActivationFunctionType.Exp

FP8_SCORES = os.environ.get("FP8_SCORES", "1") == "1"

_COMPILED = None


def _build():
    nc = bacc.Bacc("TRN2", target_bir_lowering=False, debug=False,
                   num_devices=N_CORES)

    xT = nc.dram_tensor("xT", [C, T], bf16, kind="ExternalInput").ap()
    wt = nc.dram_tensor("wt", [C, 3 * CH], bf16, kind="ExternalInput").ap()
    wpt = nc.dram_tensor("wpt", [CH, C], bf16, kind="ExternalInput").ap()
    bqk = nc.dram_tensor("bqk", [128, 4], f32, kind="ExternalInput").ap()
    bvb2 = nc.dram_tensor("bvb2", [128, 512], f32, kind="ExternalInput").ap()
    Sm = nc.dram_tensor("Sm", [128, 512], bf16, kind="ExternalInput").ap()
    ident = nc.dram_tensor("ident", [128, 128], bf16, kind="ExternalInput").ap()
    out = nc.dram_tensor("out_partial", [T, C], bf16, kind="ExternalOutput").ap()

    NT512 = T // 512          # 4   512-token stripes
    NT128 = T // 128          # 16  128-token tiles
    NC128 = C // 128          # 8   contraction tiles

    kdt = fp8 if FP8_SCORES else bf16

    xT_r = xT.rearrange("(o p) t -> p o t", p=128)
    wt_r = wt.rearrange("(o p) f -> p o f", p=128)

    with tile.TileContext(nc) as tc:
        with tc.tile_pool(name="consts", bufs=1) as consts, \
             tc.tile_pool(name="qkv", bufs=1) as qkv, \
             tc.tile_pool(name="xp", bufs=4) as xp, \
             tc.tile_pool(name="pp", bufs=11) as pp, \
             tc.tile_pool(name="op", bufs=4) as op, \
             tc.tile_pool(name="small", bufs=4) as small, \
             tc.tile_pool(name="ps_qkv", bufs=2, space="PSUM") as ps_qkv, \
             tc.tile_pool(name="ps_big", bufs=2, space="PSUM") as ps_big, \
             tc.tile_pool(name="ps_pv", bufs=2, space="PSUM") as ps_pv:

            # ---- constants; the first chains gate only on the minimal
            #      fj0/fj2 weight slices (1KB each) + x stripe 0 ----
            wt_sb = consts.tile([128, NC128, 3 * CH], bf16)
            xt0 = xp.tile([128, NC128, 512], bf16, tag="xt")
            nc.sync.dma_start(wt_sb[:, :, 0:256], wt_r[:, :, 0:256])
            nc.sync.dma_start(xt0[:], xT_r[:, :, ts(0, 512)])
            bqk_sb = consts.tile([128, 4], f32)
            nc.sync.dma_start(bqk_sb[:], bqk)
            nc.sync.dma_start(wt_sb[:, :, 256:512], wt_r[:, :, 256:512])
            nc.sync.dma_start(wt_sb[:, :, 512:], wt_r[:, :, 512:])
            bvb_sb = consts.tile([128, 2, 4, 64], f32)
            nc.sync.dma_start(
                bvb_sb[:], bvb2.rearrange("p (m h d) -> p m h d", m=2, h=4))
            S4_sb = consts.tile([128, 4, 128], bf16)
            nc.sync.dma_start(S4_sb[:], Sm.rearrange("p (h c) -> p h c", h=4))
            id_sb = consts.tile([128, 128], bf16)
            nc.sync.dma_start(id_sb[:], ident)
            xts = [xt0]
            for ti in range(1, NT512):
                xt = xp.tile([128, NC128, 512], bf16, tag="xt")
                nc.sync.dma_start(xt[:], xT_r[:, :, ts(ti, 512)])
                xts.append(xt)

            # ---- persistent activations ----
            # q/k live in bf16 [part=2heads*64d, slab, t]; stripes >= 1 also
            # get an fp8 copy in DoubleRow layout [part, slot, slab, t] whose
            # slot 1 is all-zero, so the fp8 DoubleRow pair sums to k.T q.
            # Stripe 0 (short attention rows, no error averaging) keeps its
            # scores in bf16.
            # bf16 q/k only needed for stripe 0's scores when fp8 is on
            qkT = 512 if FP8_SCORES else T
            qT_bf = qkv.tile([128, 2, qkT], bf16)
            kT_bf = qkv.tile([128, 2, qkT], bf16)
            vaug = qkv.tile([128, NT128, HPC, D + 1], bf16)  # [tok, ti, h, d|1]
            y_sb = qkv.tile([128, 2, NT128, 128], bf16)      # [tok, slab, tg, ch']
            yT_sb = qkv.tile([128, 2, T], bf16)              # [ch', slab, tok]
            if FP8_SCORES:
                qT8 = qkv.tile([128, 2, 2, T], fp8)
                kT8 = qkv.tile([128, 2, 2, T], fp8)

            # PE warmup: keep the tensor engine continuously busy from t~0.5us
            # until the first real chains, so the p-state ramp completes and
            # never resets. Uses the pv psum ring, whose first real use is far
            # later, so the ring slot is free again in time.
            wu = consts.tile([128, 512], bf16)
            nc.vector.memset(wu[:], 0.0)
            wu_ps = ps_pv.tile([128, HPC, D + 1], f32, tag="pv")
            for _ in range(28):
                nc.tensor.matmul(wu_ps[:].rearrange("p h d -> p (h d)"),
                                 wu[:, :128], wu[:, :HPC * (D + 1)],
                                 start=True, stop=True)

            if FP8_SCORES:
                nc.gpsimd.memset(qT8[:, 1], 0.0)
                nc.gpsimd.memset(kT8[:, 1], 0.0)
            nc.vector.memset(vaug[:, :, :, D:D + 1], 1.0)

            wpt_sb = consts.tile([128, 2, C], bf16)
            nc.sync.dma_start(
                wpt_sb[:], wpt.rearrange("(s p) o -> p s o", p=128))

            def qkv_fj(ti, fj, src, slab):
                # stripes >= 1 evacuate straight to the fp8 DoubleRow copy;
                # stripe 0 (bf16 scores) evacuates to the bf16 tiles.
                xt = xts[ti]
                ps = ps_qkv.tile([128, 512], f32, tag="qkv")
                for ci in range(NC128):
                    nc.tensor.matmul(
                        ps[:], wt_sb[:, ci, ts(fj, 128)], xt[:, ci, :],
                        start=(ci == 0), stop=(ci == NC128 - 1))
                if FP8_SCORES and ti > 0:
                    dst8 = qT8 if src is qT_bf else kT8
                    dest = dst8[:, 0, slab, ts(ti, 512)]
                else:
                    dest = src[:, slab, ts(ti, 512)]
                nc.vector.tensor_add(
                    out=dest, in0=ps[:],
                    in1=bqk_sb[:, fj:fj + 1].to_broadcast([128, 512]))

            def qkv_v(ti):
                xt = xts[ti]
                for m in range(2):
                    pv2 = ps_qkv.tile([128, 2, 4, 64], f32, tag="qkv")
                    for tj in range(2):
                        for ci in range(NC128):
                            nc.tensor.matmul(
                                pv2[:, tj], xt[:, ci, ts(2 * m + tj, 128)],
                                wt_sb[:, ci, 512:512 + CH],
                                start=(ci == 0), stop=(ci == NC128 - 1))
                    nc.vector.tensor_add(
                        out=vaug[:, 4 * ti + 2 * m:4 * ti + 2 * m + 2, :, 0:D],
                        in0=pv2[:], in1=bvb_sb[:])

            def conv_fp8(ti, src, slab):
                # stripe-0 k also needs an fp8 copy for later stripes' scores;
                # it is off the critical path, so GpSimd does the cast.
                if not FP8_SCORES or ti > 0 or src is qT_bf:
                    return
                nc.gpsimd.tensor_copy(
                    kT8[:, 0, slab, ts(ti, 512)], src[:, slab, ts(ti, 512)])

            def qk_stripe(ti):
                # q slab 0 first: the next stripe's first scores gate on it
                for fj, src, slab in ((0, qT_bf, 0), (1, kT_bf, 0),
                                      (2, qT_bf, 1), (3, kT_bf, 1)):
                    qkv_fj(ti, fj, src, slab)
                    conv_fp8(ti, src, slab)

            def emit_pair(qi, pi, p_t, heads):
                """Scores + exp for pair pi of stripe qi, given heads only.
                Masks are NOT emitted here (need all 4 heads done)."""
                use8 = FP8_SCORES and qi > 0
                diag = pi >= 2 * qi
                j0 = 2 * pi - 4 * qi
                if diag:
                    w0 = 512 - 128 * j0
                    offs, q0s = (0, w0), (128 * j0, 128 * (j0 + 1))
                    wtot = w0 + (384 - 128 * j0)
                else:
                    offs, q0s, wtot = (0, 512), (0, 0), 1024
                for h in heads:
                    hp, hs = (h % 2) * D, h // 2
                    psc = ps_big.tile([128, 1024], f32, tag="big")
                    for half in range(2):
                        ki = 2 * pi + half
                        w = 512 - q0s[half]
                        if use8:
                            nc.tensor.matmul(
                                psc[:, ds(offs[half], w)],
                                kT8[hp:hp + D, :, hs, ts(ki, 128)],
                                qT8[hp:hp + D, :, hs,
                                    ds(512 * qi + q0s[half], w)],
                                start=True, stop=True, perf_mode=DR)
                        else:
                            nc.tensor.matmul(
                                psc[:, ds(offs[half], w)],
                                kT_bf[hp:hp + D, hs, ts(ki, 128)],
                                qT_bf[hp:hp + D, hs,
                                      ds(512 * qi + q0s[half], w)],
                                start=True, stop=True)
                    nc.scalar.activation(
                        p_t[:, h, 0:wtot], psc[:, 0:wtot], Exp, scale=0.125)
                return offs if diag else None

            def emit_mask(p_t, offs, hgs=(0, 1)):
                # split per head-pair so late chains unblock as soon as their
                # heads' exps land
                for hg in hgs:
                    for half in range(2):
                        nc.vector.tensor_mul(
                            out=p_t[:, 2 * hg:2 * hg + 2, ds(offs[half], 128)],
                            in0=p_t[:, 2 * hg:2 * hg + 2, ds(offs[half], 128)],
                            in1=S4_sb[:, 2 * hg:2 * hg + 2, :])

            # stripe 0 fast path: interleave its (bf16) pair-0 scores with the
            # qkv chains so the ACT pipeline starts as early as possible.
            p0_t = pp.tile([128, HPC, 1024], bf16, tag="p")
            qkv_fj(0, 0, qT_bf, 0)
            qkv_fj(0, 1, kT_bf, 0)
            conv_fp8(0, kT_bf, 0)
            with tc.high_priority(offset=8000):
                offs0 = emit_pair(0, 0, p0_t, (0, 1))
                emit_mask(p0_t, offs0, hgs=(0,))
            qkv_fj(0, 2, qT_bf, 1)
            qkv_fj(0, 3, kT_bf, 1)
            conv_fp8(0, kT_bf, 1)
            with tc.high_priority(offset=8000):
                emit_pair(0, 0, p0_t, (2, 3))
                emit_mask(p0_t, offs0, hgs=(1,))

            for ti in range(NT512):
                # ---------- attention stripe qi = ti ----------
                # p tiles use a packed column layout [128, h, 1024]: for
                # off-diagonal ki-pairs, cols [512*half : 512*half+512]; for
                # diagonal pairs the two ragged blocks [q0:512] are packed
                # back-to-back at [0:w0] and [w0:w0+w1] so exp is one
                # activation per (pair, head).
                qi = ti
                nk = 4 * qi + 4
                npair = nk // 2

                def pcol(ki, qq):
                    # packed p-tile column of query subtile qq for key tile ki
                    j = ki - 4 * qi
                    if j < 0:
                        return 512 * (ki % 2) + 128 * qq
                    half = j % 2
                    off = 0 if half == 0 else 512 - 128 * (j - 1)
                    return off + 128 * (qq - j)

                # scores/exp outrank all other ready work: they are cheap on
                # PE but gate the ACT stream, which is the secondary
                # bottleneck.
                p_tiles = []
                with tc.high_priority(offset=8000):
                    for pi in range(npair):
                        if qi == 0 and pi == 0:
                            p_tiles.append(p0_t)
                            continue
                        p_t = pp.tile([128, HPC, 1024], bf16, tag="p")
                        p_tiles.append(p_t)
                        offs = emit_pair(qi, pi, p_t, (0, 1))
                        if offs is not None:
                            emit_mask(p_t, offs, hgs=(0,))
                        emit_pair(qi, pi, p_t, (2, 3))
                        if offs is not None:
                            emit_mask(p_t, offs, hgs=(1,))

                # q/k for the next stripe gate the next ACT phase, so they
                # go ahead of this stripe's pv/outproj filler work; the next
                # v chains are pure filler and go after the pv chains.
                if ti + 1 < NT512:
                    qk_stripe(ti + 1)
                qkv_v(ti)

                pvs = [ps_pv.tile([128, HPC, D + 1], f32, tag="pv",
                                  name="pv")]
                for qq in range(4):
                    pv = pvs[qq]
                    lastki = 4 * qi + qq
                    for h in range(HPC):
                        for ki in range(lastki + 1):
                            nc.tensor.matmul(
                                pv[:, h],
                                p_tiles[ki // 2][:, h, ds(pcol(ki, qq), 128)],
                                vaug[:, ki, h, :],
                                start=(ki == 0), stop=(ki == lastki))
                    rec = small.tile([128, HPC, 1], f32, tag="rec")
                    nc.vector.reciprocal(rec[:], pv[:, :, D:D + 1])
                    # allocate the next qq's psum slot before the transposes
                    # claim ring positions, so its chains start a cascade step
                    # earlier
                    if qq + 1 < 4:
                        pvs.append(ps_pv.tile([128, HPC, D + 1], f32,
                                              tag="pv", name="pv"))
                    g = 4 * qi + qq
                    nc.vector.tensor_mul(
                        out=y_sb[:, :, g, :].rearrange(
                            "p s (e d) -> p s e d", e=2),
                        in0=pv[:, :, 0:D].rearrange(
                            "p (s e) d -> p s e d", s=2),
                        in1=rec[:].rearrange("p (s e) o -> p s e o", s=2)
                            .to_broadcast([128, 2, 2, D]))
                    # reuse the (dead after normalize) pv psum tile for the
                    # transpose output instead of a fresh ring slot, so the
                    # next qq's chains never wait on this qq's cascade
                    ytr = pv.bitcast(bf16)
                    for s in range(2):
                        nc.tensor.transpose(
                            ytr[:, s, 0:128], y_sb[:, s, g, :], id_sb[:])
                        nc.vector.tensor_copy(
                            yT_sb[:, s, ts(g, 128)], ytr[:, s, 0:128])

                # ---- output projection, deferred one stripe so its PE/DVE
                #      work fills the ACT-bound phases of later stripes ----
                def outproj_group(st):
                    last = st == NT512 - 1
                    for g in range(4 * st, 4 * st + 4):
                        ot = op.tile([128, 1024], bf16, tag="ot")
                        for oi in range(2):
                            po = ps_qkv.tile([128, 512], f32, tag="qkv")
                            for s in range(2):
                                nc.tensor.matmul(
                                    po[:], yT_sb[:, s, ts(g, 128)],
                                    wpt_sb[:, s, ts(oi, 512)],
                                    start=(s == 0), stop=(s == 1))
                            if last and oi == 1:
                                # ACT is idle in the kernel tail; splitting the
                                # evacuation across engines shortens the final
                                # cascade
                                nc.scalar.activation(
                                    ot[:, ts(oi, 512)], po[:],
                                    mybir.ActivationFunctionType.Copy)
                            else:
                                nc.vector.tensor_copy(ot[:, ts(oi, 512)], po[:])
                            if last:
                                nc.sync.dma_start(
                                    out[ts(g, 128), ts(oi, 512)],
                                    ot[:, ts(oi, 512)])
                        if not last:
                            nc.sync.dma_start(out[ts(g, 128), :], ot[:])
                if ti >= 1:
                    outproj_group(ti - 1)
                if ti == NT512 - 1:
                    outproj_group(ti)

    nc.compile()
    return nc


def _get_compiled():
    global _COMPILED
    if _COMPILED is None:
        _COMPILED = _build()
    return _COMPILED


def _host_prep(x, W_attn, b_attn, W_proj, b_proj):
    xTb = [np.ascontiguousarray(x[b].T).astype(ml_dtypes.bfloat16)
           for b in range(B)]
    tri = (np.arange(128, dtype=np.int32)[None, :]
           >= np.arange(128, dtype=np.int32)[:, None])
    Sm = np.ascontiguousarray(
        np.tile(tri, (1, 4))).astype(ml_dtypes.bfloat16)
    in_maps = []
    for c in range(N_CORES):
        b, g = divmod(c, 4)
        ch = slice(CH * g, CH * (g + 1))
        Wq = W_attn[ch]
        Wk = W_attn[C:][ch]
        Wv = W_attn[2 * C:][ch]
        wt_c = np.ascontiguousarray(
            np.concatenate([Wq[:128], Wk[:128], Wq[128:], Wk[128:], Wv],
                           axis=0).T).astype(ml_dtypes.bfloat16)
        bq = b_attn[ch]
        bk = b_attn[C:][ch]
        bv = b_attn[2 * C:][ch]
        bqk_c = np.ascontiguousarray(
            np.concatenate([bq[:128], bk[:128], bq[128:], bk[128:]])
            .reshape(4, 128).T).astype(np.float32)
        bvb2_c = np.ascontiguousarray(
            np.tile(np.broadcast_to(bv[None, :], (128, CH)),
                    (1, 2))).astype(np.float32)
        wpt_c = np.ascontiguousarray(
            W_proj[:, ch].T).astype(ml_dtypes.bfloat16)
        in_maps.append({
            "xT": xTb[b],
            "wt": wt_c,
            "wpt": wpt_c,
            "bqk": bqk_c,
            "bvb2": bvb2_c,
            "Sm": Sm,
            "ident": np.eye(128, dtype=ml_dtypes.bfloat16),
        })
    return in_maps


def kernel(x, W_attn, b_attn, W_proj, b_proj):
    x = np.asarray(x, dtype=np.float32)
    W_attn = np.asarray(W_attn, dtype=np.float32)
    b_attn = np.asarray(b_attn, dtype=np.float32)
    W_proj = np.asarray(W_proj, dtype=np.float32)
    b_proj = np.asarray(b_proj, dtype=np.float32)

    nc = _get_compiled()
    in_maps = _host_prep(x, W_attn, b_attn, W_proj, b_proj)
    res = run_bass_kernel_spmd(nc, in_maps, core_ids=list(range(N_CORES)))

    out = np.empty((B, T, C), dtype=np.float32)
    for b in range(B):
        acc = np.asarray(res.results[4 * b]["out_partial"]).astype(np.float32)
        for g in range(1, 4):
            acc += np.asarray(
                res.results[4 * b + g]["out_partial"]).astype(np.float32)
        out[b] = acc + b_proj
    return out
